# revision 16
# baseline (speedup 1.0000x reference)
"""Trainium2 Bass kernel for nn_ConcatLSTMLayer.

Math (per reference):
    md       = date_contexts @ W_mh.T                      [B, 4H]
    x_gates  = einsum(input, W_ih) + b_ih + md + b_hh      [T, B, 4H]
    per step: gates = x_gates[t] + h @ W_hh.T; i,f,g,o = split(gates)
              c = sig(f)*c + sig(i)*tanh(g); h = sig(o)*tanh(c)
    returns (outputs[T,B,H], h_T, c_T)

Sharding: data-parallel over batch B=64 across 8 cores (8 rows each),
weights replicated.  All matmul FLOPs and the recurrence run on device.

Device layout (per core, batch slice of BL=8):
  Phase 1: xg^T[4H, (t,b)] = [W_ih | W_mh]^T-matmul over K=I+C=640,
           accumulated in PSUM, + bias, stored fp16 in SBUF
           as xg_sb[128p, 16m, T*BL] (p = 4H row within 128-chunk m).
  Phase 2: 512 sequential steps.  gates^T computed as 64 matmuls
           (W_hh^T tiles [128,128] stationary fp16 -> FWL, h moving
           [128, 8]) into 4 PSUM group tiles (i,f,g,o), then
           DVE add xg + ACT sigmoid/tanh + DVE cell update.
"""

import numpy as np

T, B, I, H, C = 512, 64, 512, 512, 128
NCORES = 8
BL = B // NCORES          # 8 batch rows per core
G4 = 4 * H                # 2048
KH = H // 128             # 4  K-chunks for W_hh
C2 = 2 * C                # context features padded: [dc | 1 | 0...] so the
                          # bias lands in the matmul as an extra K row
KA = (I + C2) // 128      # 6  K-chunks for [W_ih | W_mh | bias | 0]
MCH = G4 // 128           # 16 M-chunks of gates
TCH = 64                  # timesteps per phase-1 chunk
NT = T // TCH             # 8 phase-1 chunks
NB1 = TCH * BL            # 512 = phase-1 matmul free dim

_cache = {}


def _build(t_steps):
    import concourse.bass as bass
    import concourse.mybir as mybir
    import concourse.tile as tile
    from concourse import bacc

    F32 = mybir.dt.float32
    F16 = mybir.dt.float16
    SIG = mybir.ActivationFunctionType.Sigmoid
    TANH = mybir.ActivationFunctionType.Tanh

    nt = max(1, t_steps // TCH)

    nc = bacc.Bacc("TRN2", target_bir_lowering=False, debug=False, num_devices=NCORES)
    x16_e = nc.dram_tensor("x16", [t_steps, BL, I], F16, kind="ExternalInput")
    dcrep_e = nc.dram_tensor("dcrep", [C2, NB1], F16, kind="ExternalInput")
    whhT_e = nc.dram_tensor("whhT", [H, G4], F16, kind="ExternalInput")
    waugT_e = nc.dram_tensor("waugT", [I + C2, G4], F16, kind="ExternalInput")
    h0T16_e = nc.dram_tensor("h0T16", [H, BL], F16, kind="ExternalInput")
    c0T_e = nc.dram_tensor("c0T", [H, BL], F32, kind="ExternalInput")
    out_e = nc.dram_tensor("out", [t_steps, BL, H], F32, kind="ExternalOutput")
    hT_e = nc.dram_tensor("hT", [BL, H], F32, kind="ExternalOutput")
    cT_e = nc.dram_tensor("cT", [BL, H], F32, kind="ExternalOutput")

    with tile.TileContext(nc) as tc:
        with (
            tc.tile_pool(name="persist", bufs=1) as persist,
            tc.tile_pool(name="small", bufs=2) as small,
        ):
            whh_sb = persist.tile([128, KH, G4], F16, name="whh_sb")
            nc.sync.dma_start(
                out=whh_sb, in_=whhT_e.rearrange("(k p) m -> p k m", p=128)
            )
            waug_sb = persist.tile([128, KA, G4], F16, name="waug_sb")
            nc.sync.dma_start(
                out=waug_sb, in_=waugT_e.rearrange("(k p) m -> p k m", p=128)
            )
            dcrep_sb = persist.tile([128, 2, NB1], F16, name="dcrep_sb")
            nc.sync.dma_start(
                out=dcrep_sb, in_=dcrep_e.rearrange("(c p) n -> p c n", p=128)
            )
            # all gate pre-activations, fp16, entirely SBUF-resident
            xg_sb = persist.tile([128, MCH, t_steps * BL], F16, name="xg_sb")
            tc.strict_bb_all_engine_barrier()

            # ---------------- Phase 1: xg precompute ----------------
            with (
                tc.tile_pool(name="p1x", bufs=2) as p1x,
                tc.tile_pool(name="p1ps", bufs=4, space="PSUM") as p1ps,
            ):
                for cch in range(nt):
                    xt = []
                    for k in range(4):
                        xk = p1x.tile(
                            [128, TCH, BL], F16, tag=f"x{k}", name=f"x{k}_{cch}"
                        )
                        nc.sync.dma_start(
                            out=xk,
                            in_=x16_e[
                                cch * TCH : (cch + 1) * TCH, :, k * 128 : (k + 1) * 128
                            ].rearrange("t b i -> i t b"),
                        )
                        xt.append(xk.rearrange("p t b -> p (t b)"))
                    xt.append(dcrep_sb[:, 0, :])
                    xt.append(dcrep_sb[:, 1, :])
                    for m in range(MCH):
                        ps1 = p1ps.tile([128, NB1], F32, tag="p1", name=f"p1_{cch}_{m}")
                        for k in range(KA):
                            nc.tensor.matmul(
                                ps1,
                                lhsT=waug_sb[:, k, 128 * m : 128 * (m + 1)],
                                rhs=xt[k],
                                start=(k == 0),
                                stop=(k == KA - 1),
                            )
                        nc.vector.tensor_copy(
                            out=xg_sb[:, m, NB1 * cch : NB1 * (cch + 1)],
                            in_=ps1,
                        )

            # ---------------- Phase 2: recurrence ----------------
            with tc.tile_pool(name="p2ps", bufs=1, space="PSUM") as p2ps:
                h16 = small.tile([128, KH, BL], F16, tag="h16", name="h16_init")
                nc.sync.dma_start(
                    out=h16, in_=h0T16_e.rearrange("(k p) b -> p k b", p=128)
                )
                c_prev = small.tile([128, KH, BL], F32, tag="c", name="c_init")
                nc.sync.dma_start(
                    out=c_prev, in_=c0T_e.rearrange("(k p) b -> p k b", p=128)
                )
                tc.strict_bb_all_engine_barrier()
                OB = 8  # output steps batched per DMA
                h32w = None
                for t in range(t_steps):
                    if t % OB == 0:
                        # wide ring tile: [p, step-in-group, b, k]; the whole
                        # group is stored with ONE merged-[128,256] DMA
                        h32w = small.tile(
                            [128, OB, BL, KH], F32, tag="h32w", name=f"h32w_{t}"
                        )
                    ps = [
                        p2ps.tile([128, 4, BL], F32, tag=f"ps{g}", name=f"ps{g}_{t}")
                        for g in range(4)
                    ]
                    # sweep order g,i,f,o: the tanh(g)->c chain starts earliest,
                    # only sigmoid(o) is exposed after the sweep
                    for g in (2, 0, 1, 3):
                        for j in range(4):
                            m = 4 * g + j
                            for k in range(KH):
                                nc.tensor.matmul(
                                    ps[g][:, j, :],
                                    lhsT=whh_sb[:, k, 128 * m : 128 * (m + 1)],
                                    rhs=h16[:, k, :],
                                    start=(k == 0),
                                    stop=(k == KH - 1),
                                )
                    gact = [None] * 4
                    for g in (2, 0, 1, 3):
                        gs = small.tile(
                            [128, 4, BL], F32, tag=f"gs{g}", name=f"gs{g}_{t}"
                        )
                        nc.vector.tensor_add(
                            gs, ps[g], xg_sb[:, 4 * g : 4 * (g + 1), BL * t : BL * (t + 1)]
                        )
                        ga = small.tile(
                            [128, 4, BL], F32, tag=f"ga{g}", name=f"ga{g}_{t}"
                        )
                        nc.scalar.activation(ga, gs, TANH if g == 2 else SIG)
                        gact[g] = ga
                    t1 = small.tile([128, 4, BL], F32, tag="t1", name=f"t1_{t}")
                    nc.vector.tensor_mul(t1, gact[0], gact[2])
                    c_new = small.tile([128, 4, BL], F32, tag="c", name=f"c_{t}")
                    nc.vector.tensor_mul(c_new, c_prev, gact[1])
                    nc.vector.tensor_add(c_new, c_new, t1)
                    tc_t = small.tile([128, 4, BL], F32, tag="tc", name=f"tc_{t}")
                    nc.scalar.activation(tc_t, c_new, TANH)
                    h16 = small.tile([128, KH, BL], F16, tag="h16", name=f"h16_{t}")
                    nc.vector.tensor_mul(h16, gact[3], tc_t)
                    nc.vector.tensor_mul(
                        h32w[:, t % OB].rearrange("p b k -> p k b"), gact[3], tc_t
                    )
                    if t % OB == OB - 1:
                        t0g = t - (OB - 1)
                        nc.sync.dma_start(
                            out=out_e[t0g : t0g + OB].rearrange(
                                "t b (k p) -> p (t b k)", p=128
                            ),
                            in_=h32w,
                        )
                    c_prev = c_new
                nc.sync.dma_start(
                    out=hT_e.rearrange("b (k p) -> p (b k)", p=128),
                    in_=h32w[:, (t_steps - 1) % OB],
                )
                cT32 = small.tile([128, BL, KH], F32, tag="ct32", name="cT32")
                nc.vector.tensor_copy(cT32.rearrange("p b k -> p k b"), c_prev)
                nc.sync.dma_start(
                    out=cT_e.rearrange("b (k p) -> p (b k)", p=128), in_=cT32
                )
    nc.compile()
    return nc


def _get_nc(t_steps):
    if t_steps not in _cache:
        _cache[t_steps] = _build(t_steps)
    return _cache[t_steps]


def _prep_inputs(input, date_contexts, h0, c0, weight_ih, weight_hh, weight_mh,
                 bias_ih, bias_hh):
    x = np.asarray(input, np.float32)
    t_steps = x.shape[0]
    bias = (np.asarray(bias_ih) + np.asarray(bias_hh)).astype(np.float32)
    # augmented weights: K rows = [W_ih cols | W_mh cols | bias | zero-pad]
    waug = np.zeros((G4, I + C2), np.float32)
    waug[:, :I] = np.asarray(weight_ih)
    waug[:, I : I + C] = np.asarray(weight_mh)
    waug[:, I + C] = bias
    waugT = np.ascontiguousarray(waug.T.astype(np.float16))
    whhT = np.ascontiguousarray(np.asarray(weight_hh).T.astype(np.float16))
    in_maps = []
    for j in range(NCORES):
        bsl = slice(BL * j, BL * (j + 1))
        # context features per (t,b) column: [dc_b | 1 | 0...], repeated over t
        dcf = np.zeros((C2, BL), np.float16)
        dcf[:C] = np.asarray(date_contexts)[bsl].T.astype(np.float16)
        dcf[C] = 1.0
        dcrep = np.ascontiguousarray(
            np.broadcast_to(dcf[:, None, :], (C2, TCH, BL)).reshape(C2, NB1)
        )
        in_maps.append(
            dict(
                x16=np.ascontiguousarray(x[:, bsl, :]).astype(np.float16),
                dcrep=dcrep,
                whhT=whhT,
                waugT=waugT,
                h0T16=np.ascontiguousarray(np.asarray(h0)[bsl].T).astype(np.float16),
                c0T=np.ascontiguousarray(np.asarray(c0)[bsl].T.astype(np.float32)),
            )
        )
    return t_steps, in_maps


def run_sharded(inputs, trace=False, trace_kwargs=None):
    """Shard, run on 8 cores, gather.  Returns ((out, hT, cT), BassKernelResults)."""
    from concourse.bass_utils import run_bass_kernel_spmd

    t_steps, in_maps = _prep_inputs(**inputs)
    nc = _get_nc(t_steps)
    res = run_bass_kernel_spmd(
        nc,
        in_maps,
        list(range(NCORES)),
        trace=trace,
        **(trace_kwargs or {}),
    )
    out = np.empty((t_steps, B, H), np.float32)
    hT = np.empty((B, H), np.float32)
    cT = np.empty((B, H), np.float32)
    for j in range(NCORES):
        r = res.results[j]
        out[:, BL * j : BL * (j + 1), :] = r["out"]
        hT[BL * j : BL * (j + 1)] = r["hT"]
        cT[BL * j : BL * (j + 1)] = r["cT"]
    return (out, hT, cT), res


def kernel(**inputs):
    (out, hT, cT), _ = run_sharded(inputs, trace=False)
    return out, hT, cT


# revision 20
# speedup vs baseline: 1.4008x; 1.4008x over previous
"""Trainium2 Bass kernel for nn_ConcatLSTMLayer.

Math (per reference):
    md       = date_contexts @ W_mh.T                      [B, 4H]
    x_gates  = einsum(input, W_ih) + b_ih + md + b_hh      [T, B, 4H]
    per step: gates = x_gates[t] + h @ W_hh.T; i,f,g,o = split(gates)
              c = sig(f)*c + sig(i)*tanh(g); h = sig(o)*tanh(c)
    returns (outputs[T,B,H], h_T, c_T)

Sharding: data-parallel over batch B=64 across 8 cores (8 rows each),
weights replicated.  All matmul FLOPs and the recurrence run on device.

Device layout (per core, batch slice of BL=8):
  Phase 1: xg^T[4H, (t,b)] = [W_ih | W_mh]^T-matmul over K=I+C=640,
           accumulated in PSUM, + bias, stored fp16 in SBUF
           as xg_sb[128p, 16m, T*BL] (p = 4H row within 128-chunk m).
  Phase 2: 512 sequential steps.  gates^T computed as 64 matmuls
           (W_hh^T tiles [128,128] stationary fp16 -> FWL, h moving
           [128, 8]) into 4 PSUM group tiles (i,f,g,o), then
           DVE add xg + ACT sigmoid/tanh + DVE cell update.
"""

import numpy as np

T, B, I, H, C = 512, 64, 512, 512, 128
NCORES = 8
BL = B // NCORES          # 8 batch rows per core
G4 = 4 * H                # 2048
KH = H // 128             # 4  K-chunks for W_hh
C2 = 2 * C                # context features padded: [dc | 1 | 0...] so the
                          # bias lands in the matmul as an extra K row
KA = (I + C2) // 128      # 6  K-chunks for [W_ih | W_mh | bias | 0]
MCH = G4 // 128           # 16 M-chunks of gates
TCH = 64                  # timesteps per phase-1 chunk
NT = T // TCH             # 8 phase-1 chunks
NB1 = TCH * BL            # 512 = phase-1 matmul free dim

_cache = {}


def _build(t_steps):
    import concourse.bass as bass
    import concourse.mybir as mybir
    import concourse.tile as tile
    from concourse import bacc

    F32 = mybir.dt.float32
    F16 = mybir.dt.float16
    SIG = mybir.ActivationFunctionType.Sigmoid
    TANH = mybir.ActivationFunctionType.Tanh

    nt = max(1, t_steps // TCH)

    nc = bacc.Bacc("TRN2", target_bir_lowering=False, debug=False, num_devices=NCORES)
    xT16_e = nc.dram_tensor("xT16", [I, t_steps, BL], F16, kind="ExternalInput")
    dcrep_e = nc.dram_tensor("dcrep", [C2, NB1], F16, kind="ExternalInput")
    whhT_e = nc.dram_tensor("whhT", [H, G4], F16, kind="ExternalInput")
    waugT_e = nc.dram_tensor("waugT", [I + C2, G4], F16, kind="ExternalInput")
    h0T16_e = nc.dram_tensor("h0T16", [H, BL], F16, kind="ExternalInput")
    c0T_e = nc.dram_tensor("c0T", [H, BL], F32, kind="ExternalInput")
    out_e = nc.dram_tensor("out", [t_steps, BL, H], F32, kind="ExternalOutput")
    hT_e = nc.dram_tensor("hT", [BL, H], F32, kind="ExternalOutput")
    cT_e = nc.dram_tensor("cT", [BL, H], F32, kind="ExternalOutput")

    with tile.TileContext(nc) as tc:
        with (
            tc.tile_pool(name="persist", bufs=1) as persist,
            tc.tile_pool(name="small", bufs=2) as small,
        ):
            whh_sb = persist.tile([128, KH, G4], F16, name="whh_sb")
            nc.sync.dma_start(
                out=whh_sb, in_=whhT_e.rearrange("(k p) m -> p k m", p=128)
            )
            waug_sb = persist.tile([128, KA, G4], F16, name="waug_sb")
            nc.sync.dma_start(
                out=waug_sb, in_=waugT_e.rearrange("(k p) m -> p k m", p=128)
            )
            dcrep_sb = persist.tile([128, 2, NB1], F16, name="dcrep_sb")
            nc.sync.dma_start(
                out=dcrep_sb, in_=dcrep_e.rearrange("(c p) n -> p c n", p=128)
            )
            # all gate pre-activations, fp16, entirely SBUF-resident
            xg_sb = persist.tile([128, MCH, t_steps * BL], F16, name="xg_sb")
            tc.strict_bb_all_engine_barrier()

            # ---------------- Phase 1: xg precompute ----------------
            with (
                tc.tile_pool(name="p1x", bufs=2) as p1x,
                tc.tile_pool(name="p1ps", bufs=4, space="PSUM") as p1ps,
            ):
                for cch in range(nt):
                    xt = []
                    for k in range(4):
                        xk = p1x.tile(
                            [128, TCH, BL], F16, tag=f"x{k}", name=f"x{k}_{cch}"
                        )
                        nc.sync.dma_start(
                            out=xk,
                            in_=xT16_e[
                                k * 128 : (k + 1) * 128,
                                cch * TCH : (cch + 1) * TCH,
                                :,
                            ],
                        )
                        xt.append(xk.rearrange("p t b -> p (t b)"))
                    xt.append(dcrep_sb[:, 0, :])
                    xt.append(dcrep_sb[:, 1, :])
                    for m in range(MCH):
                        ps1 = p1ps.tile([128, NB1], F32, tag="p1", name=f"p1_{cch}_{m}")
                        for k in range(KA):
                            nc.tensor.matmul(
                                ps1,
                                lhsT=waug_sb[:, k, 128 * m : 128 * (m + 1)],
                                rhs=xt[k],
                                start=(k == 0),
                                stop=(k == KA - 1),
                            )
                        nc.vector.tensor_copy(
                            out=xg_sb[:, m, NB1 * cch : NB1 * (cch + 1)],
                            in_=ps1,
                        )

            # ---------------- Phase 2: recurrence ----------------
            with tc.tile_pool(name="p2ps", bufs=1, space="PSUM") as p2ps:
                h16 = small.tile([128, KH, BL], F16, tag="h16", name="h16_init")
                nc.sync.dma_start(
                    out=h16, in_=h0T16_e.rearrange("(k p) b -> p k b", p=128)
                )
                c_prev = small.tile([128, KH, BL], F32, tag="c", name="c_init")
                nc.sync.dma_start(
                    out=c_prev, in_=c0T_e.rearrange("(k p) b -> p k b", p=128)
                )
                tc.strict_bb_all_engine_barrier()
                OB = 8  # output steps batched per DMA
                h32w = None
                for t in range(t_steps):
                    if t % OB == 0:
                        # wide ring tile: [p, step-in-group, b, k]; the whole
                        # group is stored with ONE merged-[128,256] DMA
                        h32w = small.tile(
                            [128, OB, BL, KH], F32, tag="h32w", name=f"h32w_{t}"
                        )
                    ps = [
                        p2ps.tile([128, 4, BL], F32, tag=f"ps{g}", name=f"ps{g}_{t}")
                        for g in range(4)
                    ]
                    # sweep order g,i,f,o: the tanh(g)->c chain starts earliest,
                    # only sigmoid(o) is exposed after the sweep
                    for g in (2, 0, 1, 3):
                        for j in range(4):
                            m = 4 * g + j
                            for k in range(KH):
                                nc.tensor.matmul(
                                    ps[g][:, j, :],
                                    lhsT=whh_sb[:, k, 128 * m : 128 * (m + 1)],
                                    rhs=h16[:, k, :],
                                    start=(k == 0),
                                    stop=(k == KH - 1),
                                )
                    gact = [None] * 4
                    for g in (2, 0, 1, 3):
                        gs = small.tile(
                            [128, 4, BL], F32, tag=f"gs{g}", name=f"gs{g}_{t}"
                        )
                        nc.vector.tensor_add(
                            gs, ps[g], xg_sb[:, 4 * g : 4 * (g + 1), BL * t : BL * (t + 1)]
                        )
                        ga = small.tile(
                            [128, 4, BL], F32, tag=f"ga{g}", name=f"ga{g}_{t}"
                        )
                        nc.scalar.activation(ga, gs, TANH if g == 2 else SIG)
                        gact[g] = ga
                    t1 = small.tile([128, 4, BL], F32, tag="t1", name=f"t1_{t}")
                    nc.vector.tensor_mul(t1, gact[0], gact[2])
                    c_new = small.tile([128, 4, BL], F32, tag="c", name=f"c_{t}")
                    nc.vector.tensor_mul(c_new, c_prev, gact[1])
                    nc.vector.tensor_add(c_new, c_new, t1)
                    tc_t = small.tile([128, 4, BL], F32, tag="tc", name=f"tc_{t}")
                    nc.scalar.activation(tc_t, c_new, TANH)
                    h16 = small.tile([128, KH, BL], F16, tag="h16", name=f"h16_{t}")
                    nc.vector.tensor_mul(h16, gact[3], tc_t)
                    nc.vector.tensor_mul(
                        h32w[:, t % OB].rearrange("p b k -> p k b"), gact[3], tc_t
                    )
                    # store from the ring slice on the idle SWDGE queue; the
                    # ring gives 16 steps of WAR slack so nothing blocks
                    nc.gpsimd.dma_start(
                        out=out_e[t].rearrange("b (k p) -> p (b k)", p=128),
                        in_=h32w[:, t % OB],
                    )
                    c_prev = c_new
                nc.gpsimd.dma_start(
                    out=hT_e.rearrange("b (k p) -> p (b k)", p=128),
                    in_=h32w[:, (t_steps - 1) % OB],
                )
                cT32 = small.tile([128, BL, KH], F32, tag="ct32", name="cT32")
                nc.vector.tensor_copy(cT32.rearrange("p b k -> p k b"), c_prev)
                nc.gpsimd.dma_start(
                    out=cT_e.rearrange("b (k p) -> p (b k)", p=128), in_=cT32
                )
    nc.compile()
    return nc


def _get_nc(t_steps):
    if t_steps not in _cache:
        _cache[t_steps] = _build(t_steps)
    return _cache[t_steps]


def _prep_inputs(input, date_contexts, h0, c0, weight_ih, weight_hh, weight_mh,
                 bias_ih, bias_hh):
    x = np.asarray(input, np.float32)
    t_steps = x.shape[0]
    bias = (np.asarray(bias_ih) + np.asarray(bias_hh)).astype(np.float32)
    # augmented weights: K rows = [W_ih cols | W_mh cols | bias | zero-pad]
    waug = np.zeros((G4, I + C2), np.float32)
    waug[:, :I] = np.asarray(weight_ih)
    waug[:, I : I + C] = np.asarray(weight_mh)
    waug[:, I + C] = bias
    waugT = np.ascontiguousarray(waug.T.astype(np.float16))
    whhT = np.ascontiguousarray(np.asarray(weight_hh).T.astype(np.float16))
    in_maps = []
    for j in range(NCORES):
        bsl = slice(BL * j, BL * (j + 1))
        # context features per (t,b) column: [dc_b | 1 | 0...], repeated over t
        dcf = np.zeros((C2, BL), np.float16)
        dcf[:C] = np.asarray(date_contexts)[bsl].T.astype(np.float16)
        dcf[C] = 1.0
        dcrep = np.ascontiguousarray(
            np.broadcast_to(dcf[:, None, :], (C2, TCH, BL)).reshape(C2, NB1)
        )
        in_maps.append(
            dict(
                xT16=np.ascontiguousarray(
                    x[:, bsl, :].transpose(2, 0, 1).astype(np.float16)
                ),
                dcrep=dcrep,
                whhT=whhT,
                waugT=waugT,
                h0T16=np.ascontiguousarray(np.asarray(h0)[bsl].T).astype(np.float16),
                c0T=np.ascontiguousarray(np.asarray(c0)[bsl].T.astype(np.float32)),
            )
        )
    return t_steps, in_maps


def run_sharded(inputs, trace=False, trace_kwargs=None):
    """Shard, run on 8 cores, gather.  Returns ((out, hT, cT), BassKernelResults)."""
    from concourse.bass_utils import run_bass_kernel_spmd

    t_steps, in_maps = _prep_inputs(**inputs)
    nc = _get_nc(t_steps)
    res = run_bass_kernel_spmd(
        nc,
        in_maps,
        list(range(NCORES)),
        trace=trace,
        **(trace_kwargs or {}),
    )
    out = np.empty((t_steps, B, H), np.float32)
    hT = np.empty((B, H), np.float32)
    cT = np.empty((B, H), np.float32)
    for j in range(NCORES):
        r = res.results[j]
        out[:, BL * j : BL * (j + 1), :] = r["out"]
        hT[BL * j : BL * (j + 1)] = r["hT"]
        cT[BL * j : BL * (j + 1)] = r["cT"]
    return (out, hT, cT), res


def kernel(**inputs):
    (out, hT, cT), _ = run_sharded(inputs, trace=False)
    return out, hT, cT


# revision 30
# speedup vs baseline: 4.3284x; 3.0899x over previous
"""Trainium2 Bass kernel for nn_ConcatLSTMLayer.

Math (per reference):
    md       = date_contexts @ W_mh.T                      [B, 4H]
    x_gates  = einsum(input, W_ih) + b_ih + md + b_hh      [T, B, 4H]
    per step: gates = x_gates[t] + h @ W_hh.T; i,f,g,o = split(gates)
              c = sig(f)*c + sig(i)*tanh(g); h = sig(o)*tanh(c)
    returns (outputs[T,B,H], h_T, c_T)

Sharding: data-parallel over batch B=64 across 8 cores (8 rows each),
weights replicated.  All matmul FLOPs and the recurrence run on device.

Device layout (per core, batch slice of BL=8):
  Phase 1: xg^T[4H, (t,b)] = [W_ih | W_mh]^T-matmul over K=I+C=640,
           accumulated in PSUM, + bias, stored fp16 in SBUF
           as xg_sb[128p, 16m, T*BL] (p = 4H row within 128-chunk m).
  Phase 2: 512 sequential steps.  gates^T computed as 64 matmuls
           (W_hh^T tiles [128,128] stationary fp16 -> FWL, h moving
           [128, 8]) into 4 PSUM group tiles (i,f,g,o), then
           DVE add xg + ACT sigmoid/tanh + DVE cell update.
"""

import numpy as np

T, B, I, H, C = 512, 64, 512, 512, 128
NCORES = 8
BL = B // NCORES          # 8 batch rows per core
G4 = 4 * H                # 2048
KH = H // 128             # 4  K-chunks for W_hh
C2 = 2 * C                # context features padded: [dc | 1 | 0...] so the
                          # bias lands in the matmul as an extra K row
KA = (I + C2) // 128      # 6  K-chunks for [W_ih | W_mh | bias | 0]
MCH = G4 // 128           # 16 M-chunks of gates
TCH = 64                  # timesteps per phase-1 chunk
NT = T // TCH             # 8 phase-1 chunks
NB1 = TCH * BL            # 512 = phase-1 matmul free dim

_cache = {}


def _build(t_steps):
    import concourse.bass as bass
    import concourse.mybir as mybir
    import concourse.tile as tile
    from concourse import bacc
    from concourse.masks import make_identity

    F32 = mybir.dt.float32
    F16 = mybir.dt.float16
    SIG = mybir.ActivationFunctionType.Sigmoid
    TANH = mybir.ActivationFunctionType.Tanh

    nt = max(1, t_steps // TCH)

    nc = bacc.Bacc("TRN2", target_bir_lowering=False, debug=False, num_devices=NCORES)
    xT16_e = nc.dram_tensor("xT16", [I, t_steps, BL], F16, kind="ExternalInput")
    dcrep_e = nc.dram_tensor("dcrep", [C2, NB1], F16, kind="ExternalInput")
    whhT_e = nc.dram_tensor("whhT", [H, G4], F16, kind="ExternalInput")
    waugT_e = nc.dram_tensor("waugT", [I + C2, G4], F16, kind="ExternalInput")
    h0T16_e = nc.dram_tensor("h0T16", [H, BL], F16, kind="ExternalInput")
    c0T_e = nc.dram_tensor("c0T", [H, BL], F32, kind="ExternalInput")
    out_e = nc.dram_tensor("out", [t_steps, BL, H], F32, kind="ExternalOutput")
    hT_e = nc.dram_tensor("hT", [BL, H], F32, kind="ExternalOutput")
    cT_e = nc.dram_tensor("cT", [BL, H], F32, kind="ExternalOutput")

    with tile.TileContext(nc) as tc:
        with (
            tc.tile_pool(name="persist", bufs=1) as persist,
            tc.tile_pool(name="small", bufs=2) as small,
        ):
            whh_sb = persist.tile([128, KH, G4], F16, name="whh_sb")
            nc.sync.dma_start(
                out=whh_sb, in_=whhT_e.rearrange("(k p) m -> p k m", p=128)
            )
            waug_sb = persist.tile([128, KA, G4], F16, name="waug_sb")
            nc.sync.dma_start(
                out=waug_sb, in_=waugT_e.rearrange("(k p) m -> p k m", p=128)
            )
            dcrep_sb = persist.tile([128, 2, NB1], F16, name="dcrep_sb")
            nc.sync.dma_start(
                out=dcrep_sb, in_=dcrep_e.rearrange("(c p) n -> p c n", p=128)
            )
            # all gate pre-activations, fp16, entirely SBUF-resident
            xg_sb = persist.tile([128, MCH, t_steps * BL], F16, name="xg_sb")
            ident = persist.tile([128, 128], F32, name="ident")
            make_identity(nc, ident)
            tc.strict_bb_all_engine_barrier()

            # ---------------- Phase 1: xg precompute ----------------
            with (
                tc.tile_pool(name="p1x", bufs=2) as p1x,
                tc.tile_pool(name="p1ps", bufs=2, space="PSUM") as p1ps,
            ):
                for cch in range(nt):
                    xt = []
                    for k in range(4):
                        xk = p1x.tile(
                            [128, TCH, BL], F16, tag=f"x{k}", name=f"x{k}_{cch}"
                        )
                        nc.sync.dma_start(
                            out=xk,
                            in_=xT16_e[
                                k * 128 : (k + 1) * 128,
                                cch * TCH : (cch + 1) * TCH,
                                :,
                            ],
                        )
                        xt.append(xk.rearrange("p t b -> p (t b)"))
                    xt.append(dcrep_sb[:, 0, :])
                    xt.append(dcrep_sb[:, 1, :])
                    for m in range(MCH):
                        ps1 = p1ps.tile([128, NB1], F32, tag="p1", name=f"p1_{cch}_{m}")
                        for k in range(KA):
                            nc.tensor.matmul(
                                ps1,
                                lhsT=waug_sb[:, k, 128 * m : 128 * (m + 1)],
                                rhs=xt[k],
                                start=(k == 0),
                                stop=(k == KA - 1),
                            )
                        nc.vector.tensor_copy(
                            out=xg_sb[:, m, NB1 * cch : NB1 * (cch + 1)],
                            in_=ps1,
                        )

            # ---------------- Phase 2: recurrence ----------------
            with (
                tc.tile_pool(name="p2ps", bufs=1, space="PSUM") as p2ps,
                tc.tile_pool(name="ptr", bufs=2, space="PSUM") as ptr,
            ):
                h16 = small.tile([128, KH, BL], F16, tag="h16", name="h16_init")
                nc.sync.dma_start(
                    out=h16, in_=h0T16_e.rearrange("(k p) b -> p k b", p=128)
                )
                c_prev = small.tile([128, KH, BL], F32, tag="c", name="c_init")
                nc.sync.dma_start(
                    out=c_prev, in_=c0T_e.rearrange("(k p) b -> p k b", p=128)
                )
                tc.strict_bb_all_engine_barrier()
                OB = 4  # output steps per PE-transpose group
                hhist = None
                stage = None
                for t in range(t_steps):
                    if t % OB == 0:
                        # 4 steps of h, [p, t, b, k] — one 128x128 PE transpose
                        # then a clean 512B-line DMA per group
                        hhist = small.tile(
                            [128, OB, BL, KH], F32, tag="hhist", name=f"hhist_{t}"
                        )
                    ps = [
                        p2ps.tile([128, 4, BL], F32, tag=f"ps{g}", name=f"ps{g}_{t}")
                        for g in range(4)
                    ]
                    # sweep order g,i,f,o: the tanh(g)->c chain starts earliest,
                    # only sigmoid(o) is exposed after the sweep
                    for g in (2, 0, 1, 3):
                        for j in range(4):
                            m = 4 * g + j
                            for k in range(KH):
                                nc.tensor.matmul(
                                    ps[g][:, j, :],
                                    lhsT=whh_sb[:, k, 128 * m : 128 * (m + 1)],
                                    rhs=h16[:, k, :],
                                    start=(k == 0),
                                    stop=(k == KH - 1),
                                )
                    gact = [None] * 4
                    for g in (2, 0, 1, 3):
                        gs = small.tile(
                            [128, 4, BL], F32, tag=f"gs{g}", name=f"gs{g}_{t}"
                        )
                        nc.vector.tensor_add(
                            gs, ps[g], xg_sb[:, 4 * g : 4 * (g + 1), BL * t : BL * (t + 1)]
                        )
                        ga = small.tile(
                            [128, 4, BL], F32, tag=f"ga{g}", name=f"ga{g}_{t}"
                        )
                        nc.scalar.activation(ga, gs, TANH if g == 2 else SIG)
                        gact[g] = ga
                    t1 = small.tile([128, 4, BL], F32, tag="t1", name=f"t1_{t}")
                    nc.vector.tensor_mul(t1, gact[0], gact[2])
                    c_new = small.tile([128, 4, BL], F32, tag="c", name=f"c_{t}")
                    nc.vector.tensor_mul(c_new, c_prev, gact[1])
                    nc.vector.tensor_add(c_new, c_new, t1)
                    tc_t = small.tile([128, 4, BL], F32, tag="tc", name=f"tc_{t}")
                    nc.scalar.activation(tc_t, c_new, TANH)
                    h16 = small.tile([128, KH, BL], F16, tag="h16", name=f"h16_{t}")
                    nc.vector.tensor_mul(h16, gact[3], tc_t)
                    nc.vector.tensor_mul(
                        hhist[:, t % OB].rearrange("p b k -> p k b"), gact[3], tc_t
                    )
                    if t % OB == OB - 1:
                        tr = ptr.tile([128, 128], F32, tag="tr", name=f"tr_{t}")
                        nc.tensor.transpose(
                            tr, hhist.rearrange("p t b k -> p (t b k)"), ident
                        )
                        stage = small.tile(
                            [128, 128], F32, tag="stage", name=f"stage_{t}"
                        )
                        nc.vector.tensor_copy(stage, tr)
                        t0g = t - (OB - 1)
                        nc.sync.dma_start(
                            out=bass.AP(
                                tensor=out_e,
                                offset=t0g * BL * H,
                                ap=[[128, OB * BL * KH], [1, 128]],
                            ),
                            in_=stage,
                        )
                    c_prev = c_new
                nc.sync.dma_start(
                    out=bass.AP(tensor=hT_e, offset=0, ap=[[128, BL * KH], [1, 128]]),
                    in_=stage[(OB - 1) * BL * KH :, :],
                )
                # cT: transpose c (staged in (b,k) order so the store merges)
                cbk = small.tile([128, BL, KH], F32, tag="ct32", name="cbk")
                nc.vector.tensor_copy(cbk.rearrange("p b k -> p k b"), c_prev)
                trc = ptr.tile([128, 128], F32, tag="tr", name="trc")
                nc.tensor.transpose(
                    trc[: BL * KH, :], cbk.rearrange("p b k -> p (b k)"), ident
                )
                stc = small.tile([128, 128], F32, tag="stage", name="stc")
                nc.vector.tensor_copy(stc[: BL * KH, :], trc[: BL * KH, :])
                nc.sync.dma_start(
                    out=bass.AP(tensor=cT_e, offset=0, ap=[[128, BL * KH], [1, 128]]),
                    in_=stc[: BL * KH, :],
                )
    nc.compile()
    return nc


def _get_nc(t_steps):
    if t_steps not in _cache:
        _cache[t_steps] = _build(t_steps)
    return _cache[t_steps]


def _prep_inputs(input, date_contexts, h0, c0, weight_ih, weight_hh, weight_mh,
                 bias_ih, bias_hh):
    x = np.asarray(input, np.float32)
    t_steps = x.shape[0]
    bias = (np.asarray(bias_ih) + np.asarray(bias_hh)).astype(np.float32)
    # augmented weights: K rows = [W_ih cols | W_mh cols | bias | zero-pad]
    waug = np.zeros((G4, I + C2), np.float32)
    waug[:, :I] = np.asarray(weight_ih)
    waug[:, I : I + C] = np.asarray(weight_mh)
    waug[:, I + C] = bias
    waugT = np.ascontiguousarray(waug.T.astype(np.float16))
    whhT = np.ascontiguousarray(np.asarray(weight_hh).T.astype(np.float16))
    in_maps = []
    for j in range(NCORES):
        bsl = slice(BL * j, BL * (j + 1))
        # context features per (t,b) column: [dc_b | 1 | 0...], repeated over t
        dcf = np.zeros((C2, BL), np.float16)
        dcf[:C] = np.asarray(date_contexts)[bsl].T.astype(np.float16)
        dcf[C] = 1.0
        dcrep = np.ascontiguousarray(
            np.broadcast_to(dcf[:, None, :], (C2, TCH, BL)).reshape(C2, NB1)
        )
        in_maps.append(
            dict(
                xT16=np.ascontiguousarray(
                    x[:, bsl, :].transpose(2, 0, 1).astype(np.float16)
                ),
                dcrep=dcrep,
                whhT=whhT,
                waugT=waugT,
                h0T16=np.ascontiguousarray(np.asarray(h0)[bsl].T).astype(np.float16),
                c0T=np.ascontiguousarray(np.asarray(c0)[bsl].T.astype(np.float32)),
            )
        )
    return t_steps, in_maps


def run_sharded(inputs, trace=False, trace_kwargs=None):
    """Shard, run on 8 cores, gather.  Returns ((out, hT, cT), BassKernelResults)."""
    from concourse.bass_utils import run_bass_kernel_spmd

    t_steps, in_maps = _prep_inputs(**inputs)
    nc = _get_nc(t_steps)
    res = run_bass_kernel_spmd(
        nc,
        in_maps,
        list(range(NCORES)),
        trace=trace,
        **(trace_kwargs or {}),
    )
    out = np.empty((t_steps, B, H), np.float32)
    hT = np.empty((B, H), np.float32)
    cT = np.empty((B, H), np.float32)
    for j in range(NCORES):
        r = res.results[j]
        out[:, BL * j : BL * (j + 1), :] = r["out"]
        hT[BL * j : BL * (j + 1)] = r["hT"]
        cT[BL * j : BL * (j + 1)] = r["cT"]
    return (out, hT, cT), res


def kernel(**inputs):
    (out, hT, cT), _ = run_sharded(inputs, trace=False)
    return out, hT, cT
